# revision 23
# baseline (speedup 1.0000x reference)
# DeltaFormer attention TRN2 kernel: 8-core head-parallel (2 heads/core).
# Per core: q/k/v/beta projections (fp32r matmuls), delta-rule unit-lower
# triangular solve via 128-block forward substitution (diagonal blocks
# inverted exactly with the nilpotent doubling identity), causal softmax
# attention (no running max: logits are O(5)), row-parallel output
# projection; host sums the 8 partial products.
import numpy as np

T, HID, H, D = 2048, 2048, 16, 128
NB = T // 128          # 16 t-blocks
NJ = HID // 128        # 16 contraction chunks
SCALE = float(D) ** -0.5

_NC = None


def _build():
    import concourse.bass as bass
    import concourse.bacc as bacc
    import concourse.mybir as mybir
    from concourse.tile import TileContext
    from concourse.masks import (
        make_identity,
        make_lower_triangular,
        make_upper_triangular,
    )
    from contextlib import ExitStack

    F32 = mybir.dt.float32
    F32R = mybir.dt.float32r
    BF16 = mybir.dt.bfloat16
    EXP = mybir.ActivationFunctionType.Exp
    CPY = mybir.ActivationFunctionType.Copy
    ADD = mybir.AluOpType.add
    MUL = mybir.AluOpType.mult

    nc = bacc.Bacc()
    xT_d = nc.dram_tensor("xT", [T, T], F32R, kind="ExternalInput")
    wq_d = nc.dram_tensor("wq", [HID, 2 * D], F32R, kind="ExternalInput")
    wk_d = nc.dram_tensor("wk", [HID, 2 * D], F32R, kind="ExternalInput")
    wvb_d = nc.dram_tensor("wvb", [HID, 2 * D + 2], F32R, kind="ExternalInput")
    wo_d = nc.dram_tensor("wo", [2 * D, HID], F32, kind="ExternalInput")
    bb_d = nc.dram_tensor("bb2", [1, 2], F32, kind="ExternalInput")
    y_d = nc.dram_tensor("y", [T, HID], F32, kind="ExternalOutput")
    pol_d = nc.dram_tensor("pol", [1, 2], F32, kind="ExternalOutput")

    with TileContext(nc) as tc, ExitStack() as ctx:
        consts = ctx.enter_context(tc.tile_pool(name="consts", bufs=1))
        persist = ctx.enter_context(tc.tile_pool(name="persist", bufs=1))
        pp = ctx.enter_context(tc.tile_pool(name="pp", bufs=2, space="PSUM"))
        pps = ctx.enter_context(tc.tile_pool(name="pps", bufs=3, space="PSUM"))
        ppw = ctx.enter_context(tc.tile_pool(name="ppw", bufs=2, space="PSUM"))
        ppd = ctx.enter_context(tc.tile_pool(name="ppd", bufs=1, space="PSUM"))

        # constants
        ident = consts.tile([128, 128], F32, tag="ident")
        make_identity(nc, ident)
        mSLneg = consts.tile([128, 128], F32, tag="mslneg")  # -1e30 strict lower
        make_lower_triangular(nc, mSLneg, val=-1e30, diag=False)
        mSL01 = consts.tile([128, 128], F32, tag="msl01")    # 1.0 strict lower
        make_lower_triangular(nc, mSL01, val=1.0, diag=False)
        mSU01 = consts.tile([128, 128], F32, tag="msu01")    # 1.0 strict upper
        make_upper_triangular(nc, mSU01, val=1.0, diag=False)
        ones1 = consts.tile([128, 1], BF16, tag="ones1")
        nc.vector.memset(ones1, 1.0)
        zeroW = consts.tile([128, 512], BF16, tag="zerow")
        nc.vector.memset(zeroW, 0.0)
        bbB = consts.tile([128, 2], F32, tag="bbb")
        nc.gpsimd.dma_start(out=bbB, in_=bass.AP(bb_d, 0, [[0, 128], [1, 2]]))

        # persistent tensors (per-head halves packed along free dim)
        qTt = persist.tile([128, 2 * T], BF16, tag="qT")   # [d, t] per head
        kTt = persist.tile([128, 2 * T], BF16, tag="kT")
        vt = persist.tile([128, 2 * T], F32, tag="v")      # [t, d] row blocks
        nbeta = persist.tile([128, 2 * NB], F32, tag="nbeta")  # -(beta+bb)
        polsb = persist.tile([1, 2], F32, tag="polsb")

        # ---------------- phase 1: projections ----------------
        with tc.tile_pool(name="xtp", bufs=1) as xtp, \
             tc.tile_pool(name="wtp", bufs=1) as wtp:
            xt = xtp.tile([128, NJ * T], F32R, tag="xt")
            for jc in range(NJ):
                nc.sync.dma_start(
                    out=xt[:, jc * T:(jc + 1) * T],
                    in_=xT_d[jc * 128:(jc + 1) * 128, :],
                )

            def xsl(jc, t0, tw):
                return xt[:, jc * T + t0: jc * T + t0 + tw]

            # v + beta pass
            wvbt = wtp.tile([128, NJ * 258], F32R, tag="w")
            for jc in range(NJ):
                nc.sync.dma_start(
                    out=wvbt[:, jc * 258:(jc + 1) * 258],
                    in_=wvb_d[jc * 128:(jc + 1) * 128, :])
            for tb in range(NB):
                ps = pp.tile([128, 258], F32, tag="psp")
                for jc in range(NJ):
                    nc.tensor.matmul(
                        ps, xsl(jc, tb * 128, 128),
                        wvbt[:, jc * 258:(jc + 1) * 258],
                        start=(jc == 0), stop=(jc == NJ - 1))
                for h in range(2):
                    nc.scalar.activation(
                        vt[:, h * T + tb * 128: h * T + tb * 128 + 128],
                        ps[:, h * 128:(h + 1) * 128], CPY)
                tmpb = wtp.tile([128, 2], F32, tag="tmpb", bufs=4)
                nc.vector.tensor_copy(tmpb, ps[:, 256:258])
                for h in range(2):
                    nc.vector.tensor_scalar(
                        out=nbeta[:, h * NB + tb: h * NB + tb + 1],
                        in0=tmpb[:, h:h + 1],
                        scalar1=bbB[:, h:h + 1], scalar2=-1.0,
                        op0=ADD, op1=MUL)

            # q pass (scaled by 1/sqrt(D)), then k pass
            for name, wd, dst, scl in (("q", wq_d, qTt, SCALE),
                                       ("k", wk_d, kTt, 1.0)):
                wt = wtp.tile([128, NJ * 256], F32R, tag="w")
                for jc in range(NJ):
                    nc.sync.dma_start(
                        out=wt[:, jc * 256:(jc + 1) * 256],
                        in_=wd[jc * 128:(jc + 1) * 128, :])
                for h in range(2):
                    for tch in range(4):
                        ps = pp.tile([128, 512], F32, tag="psp")
                        for jc in range(NJ):
                            nc.tensor.matmul(
                                ps,
                                wt[:, jc * 256 + h * 128: jc * 256 + h * 128 + 128],
                                xsl(jc, tch * 512, 512),
                                start=(jc == 0), stop=(jc == NJ - 1))
                        nc.scalar.activation(
                            dst[:, h * T + tch * 512: h * T + (tch + 1) * 512],
                            ps, CPY, scale=scl)

        # phase >=2 persistent tensors (alive only after xT pool is freed)
        sol = ctx.enter_context(tc.tile_pool(name="sol", bufs=1))
        nbRow = sol.tile([1, 2 * T], F32, tag="nbrow")
        accT = sol.tile([128, 2 * T], F32, tag="accT")  # [d, t]
        oT = sol.tile([128, 2 * T], F32, tag="oT")      # [d, t]
        ut = sol.tile([128, 2 * T], BF16, tag="u")      # [t, d] row blocks
        den = sol.tile([1, 2 * T], F32, tag="den")
        oTb = sol.tile([128, 2 * T], BF16, tag="oTb")   # normalized, bf16

        # negbeta row [1, 2T] via transpose + sbuf-to-sbuf dma
        for h in range(2):
            pst = pps.tile([16, 128], F32, tag="pdg")
            nc.tensor.transpose(pst, nbeta[:, h * NB:(h + 1) * NB], ident)
            nbTs = sol.tile([16, 128], F32, tag="nbts")
            nc.vector.tensor_copy(nbTs, pst)
            nc.gpsimd.dma_start(out=nbRow[0:1, h * T:(h + 1) * T], in_=nbTs)

        nc.vector.memset(accT, 0.0)
        nc.vector.memset(oT, 0.0)
        nc.vector.memset(den, 0.0)

        # ---------------- phase 2: solve + attention accumulation ----------
        with tc.tile_pool(name="ep", bufs=3) as epool, \
             tc.tile_pool(name="rawp", bufs=3) as rawpool, \
             tc.tile_pool(name="wvp", bufs=18) as wvpool, \
             tc.tile_pool(name="tfp", bufs=4) as tfpool, \
             tc.tile_pool(name="wap", bufs=4) as wapool, \
             tc.tile_pool(name="nbp", bufs=2) as nbpool:

            for j in range(NB):
                for h in range(2):
                    hb = h * T
                    jb = hb + j * 128
                    qs = qTt[:, jb:jb + 128]
                    ks = kTt[:, jb:jb + 128]

                    # diagonal qk in both orientations
                    pkq = pps.tile([128, 128], F32, tag="pdg")
                    nc.tensor.matmul(pkq, ks, qs, start=True, stop=True)
                    pqk = pps.tile([128, 128], F32, tag="pdg")
                    nc.tensor.matmul(pqk, qs, ks, start=True, stop=True)

                    # E_jj = exp(kq masked to s<=t)
                    etmp = tfpool.tile([128, 128], F32, tag="tmpf")
                    nc.vector.tensor_tensor(etmp, pkq, mSLneg, op=ADD)
                    Ejj = epool.tile([128, 128], BF16, tag="ejj")
                    nc.scalar.activation(Ejj, etmp, EXP)

                    # W0 = (-beta_t * kq)[s,t] strict upper;  V0 = W0^T
                    nbB = nbpool.tile([128, 128], F32, tag="nbb")
                    nc.gpsimd.partition_broadcast(
                        nbB, nbRow[0:1, jb:jb + 128])
                    w0f = tfpool.tile([128, 128], F32, tag="tmpf")
                    nc.vector.tensor_tensor(w0f, pkq, nbB, op=MUL)
                    Wk = [wvpool.tile([128, 128], BF16, tag="W", name=f"W{k}")
                          for k in range(7)]
                    nc.vector.tensor_tensor(Wk[0], w0f, mSU01, op=MUL)
                    v0f = tfpool.tile([128, 128], F32, tag="tmpf")
                    nc.scalar.activation(
                        v0f, pqk, CPY, scale=nbeta[:, h * NB + j: h * NB + j + 1])
                    Vprev = wvpool.tile([128, 128], BF16, tag="V")
                    nc.vector.tensor_tensor(Vprev, v0f, mSL01, op=MUL)

                    # nilpotent doubling: W_{k+1}=V_k^T W_k, V_{k+1}=W_k^T V_k
                    for k in range(6):
                        pw2 = pps.tile([128, 128], F32, tag="pdg")
                        nc.tensor.matmul(pw2, Vprev, Wk[k], start=True, stop=True)
                        nc.vector.tensor_copy(Wk[k + 1], pw2)
                        if k < 5:
                            pv2 = pps.tile([128, 128], F32, tag="pdg")
                            nc.tensor.matmul(pv2, Wk[k], Vprev, start=True, stop=True)
                            Vnew = wvpool.tile([128, 128], BF16, tag="V")
                            nc.vector.tensor_copy(Vnew, pv2)
                            Vprev = Vnew

                    # wide kq row for blocks > j (raw + exp)
                    wide = []
                    t0 = (j + 1) * 128
                    while t0 < T:
                        tw = min(512, T - t0)
                        pkw = pp.tile([128, 512], F32, tag="psp")
                        nc.tensor.matmul(
                            pkw[:, :tw], ks, qTt[:, hb + t0: hb + t0 + tw],
                            start=True, stop=True)
                        raw = rawpool.tile([128, 512], BF16, tag="raw")
                        nc.vector.tensor_copy(raw[:, :tw], pkw[:, :tw])
                        Ew = epool.tile([128, 512], BF16, tag="ew")
                        nc.scalar.activation(Ew[:, :tw], pkw[:, :tw], EXP)
                        wide.append((t0, tw, raw, Ew))
                        t0 += tw

                    # c_j = v_j - beta * acc_j   (acc transposed back to rows)
                    pat = pps.tile([128, 128], F32, tag="pdg")
                    nc.tensor.transpose(pat, accT[:, jb:jb + 128], ident)
                    ctmp = tfpool.tile([128, 128], F32, tag="tmpf")
                    nc.scalar.activation(
                        ctmp, pat, CPY, scale=nbeta[:, h * NB + j: h * NB + j + 1])
                    cf = tfpool.tile([128, 128], F32, tag="tmpf")
                    nc.vector.tensor_tensor(cf, ctmp, vt[:, jb:jb + 128], op=ADD)
                    wprev = wapool.tile([128, 128], BF16, tag="wap")
                    nc.vector.tensor_copy(wprev, cf)
                    wprevf = cf

                    # apply (I+Y)(I+Y^2)...(I+Y^64) c
                    for k in range(6, -1, -1):
                        pw = ppw.tile([128, 128], F32, tag="pw")
                        nc.tensor.matmul(pw, Wk[k], wprev, start=True, stop=True)
                        if k > 0:
                            wnf = tfpool.tile([128, 128], F32, tag="tmpf")
                            nc.vector.tensor_tensor(wnf, pw, wprevf, op=ADD)
                            wnew = wapool.tile([128, 128], BF16, tag="wap")
                            nc.vector.tensor_copy(wnew, wnf)
                            wprev, wprevf = wnew, wnf
                        else:
                            nc.vector.tensor_tensor(
                                ut[:, jb:jb + 128], pw, wprevf, op=ADD)
                    us = ut[:, jb:jb + 128]

                    # diagonal attention contributions
                    po = pps.tile([128, 128], F32, tag="pdg")
                    nc.tensor.matmul(po, us, Ejj, start=True, stop=True)
                    nc.vector.tensor_tensor(
                        oT[:, jb:jb + 128], oT[:, jb:jb + 128], po, op=ADD)
                    pd = ppd.tile([1, 512], F32, tag="pd")
                    nc.tensor.matmul(pd[:, :128], ones1, Ejj, start=True, stop=True)
                    nc.vector.tensor_tensor(
                        den[0:1, jb:jb + 128], den[0:1, jb:jb + 128],
                        pd[:, :128], op=ADD)

                    # wide updates: corrections (raw) + attention (E)
                    for (t0, tw, raw, Ew) in wide:
                        sl = slice(hb + t0, hb + t0 + tw)
                        pb = pp.tile([128, 512], F32, tag="psp")
                        nc.tensor.matmul(pb[:, :tw], us, raw[:, :tw],
                                         start=True, stop=True)
                        nc.vector.tensor_tensor(
                            accT[:, sl], accT[:, sl], pb[:, :tw], op=ADD)
                        pc = pp.tile([128, 512], F32, tag="psp")
                        nc.tensor.matmul(pc[:, :tw], us, Ew[:, :tw],
                                         start=True, stop=True)
                        nc.vector.tensor_tensor(
                            oT[:, sl], oT[:, sl], pc[:, :tw], op=ADD)
                        pd2 = ppd.tile([1, 512], F32, tag="pd")
                        nc.tensor.matmul(pd2[:, :tw], ones1, Ew[:, :tw],
                                         start=True, stop=True)
                        nc.vector.tensor_tensor(
                            den[0:1, sl], den[0:1, sl], pd2[:, :tw], op=ADD)

            # dense-softmax pollution term: u_last^T @ 0 (nan iff u overflowed)
            for h in range(2):
                pp0 = pps.tile([128, 128], F32, tag="pdg")
                nc.tensor.matmul(
                    pp0, ut[:, h * T + T - 128: h * T + T],
                    zeroW[:, :128], start=True, stop=True)
                nc.vector.tensor_copy(polsb[0:1, h:h + 1], pp0[0:1, 0:1])
            nc.sync.dma_start(out=pol_d[:, :], in_=polsb)

            # ---------------- phase 3: normalize + output projection -------
            with tc.tile_pool(name="fin", bufs=2) as fin, \
                 tc.tile_pool(name="wop", bufs=1) as wop:
                for h in range(2):
                    hb = h * T
                    nc.vector.reciprocal(
                        den[0:1, hb:hb + T], den[0:1, hb:hb + T])
                    rb = fin.tile([128, T], F32, tag="rb")
                    nc.gpsimd.partition_broadcast(rb, den[0:1, hb:hb + T])
                    nc.vector.tensor_tensor(
                        oTb[:, hb:hb + T], oT[:, hb:hb + T], rb, op=MUL)

                wos = wop.tile([128, 2 * T], F32, tag="wo")
                for h in range(2):
                    nc.sync.dma_start(
                        out=wos[:, h * T:(h + 1) * T],
                        in_=wo_d[h * 128:(h + 1) * 128, :])
                wosb = wop.tile([128, 2 * T], BF16, tag="wob")
                nc.vector.tensor_copy(wosb, wos)
                for tb in range(NB):
                    ys = fin.tile([128, HID], F32, tag="ys")
                    for nch in range(4):
                        py = pp.tile([128, 512], F32, tag="psp")
                        for h in range(2):
                            nc.tensor.matmul(
                                py,
                                oTb[:, h * T + tb * 128: h * T + tb * 128 + 128],
                                wosb[:, h * T + nch * 512: h * T + (nch + 1) * 512],
                                start=(h == 0), stop=(h == 1))
                        nc.scalar.activation(
                            ys[:, nch * 512:(nch + 1) * 512], py, CPY)
                    nc.sync.dma_start(
                        out=y_d[tb * 128:(tb + 1) * 128, :], in_=ys)

    nc.finalize()
    return nc


def _get_nc():
    global _NC
    if _NC is None:
        _NC = _build()
    return _NC


def kernel(**inputs):
    x = np.asarray(inputs["hidden_states"], np.float32)[0]
    Wq = np.asarray(inputs["Wq"], np.float32)
    Wk = np.asarray(inputs["Wk"], np.float32)
    Wv = np.asarray(inputs["Wv"], np.float32)
    Wb = np.asarray(inputs["Wb"], np.float32)
    bb = np.asarray(inputs["bb"], np.float32)
    Wo = np.asarray(inputs["Wo"], np.float32)

    xTn = np.ascontiguousarray(x.T)
    in_maps = []
    for c in range(8):
        r0 = c * 256
        in_maps.append({
            "xT": xTn,
            "wq": np.ascontiguousarray(Wq[r0:r0 + 256].T),
            "wk": np.ascontiguousarray(Wk[r0:r0 + 256].T),
            "wvb": np.ascontiguousarray(
                np.concatenate([Wv[r0:r0 + 256].T,
                                Wb[2 * c:2 * c + 2].T], axis=1)),
            "wo": np.ascontiguousarray(Wo[:, r0:r0 + 256].T),
            "bb2": np.ascontiguousarray(bb[2 * c:2 * c + 2].reshape(1, 2)),
        })

    from concourse.bass_utils import run_bass_kernel_spmd
    res = run_bass_kernel_spmd(_get_nc(), in_maps, core_ids=list(range(8)))
    globals()["_LAST_RES"] = res

    Y = np.zeros((T, HID), np.float32)
    polsum = np.float32(0.0)
    for r in res.results:
        Y += r["y"]
        polsum = polsum + r["pol"][0, 0] + r["pol"][0, 1]
    Y = Y + polsum
    return Y.reshape(1, T, HID)


# revision 30
# speedup vs baseline: 1.8355x; 1.8355x over previous
# DeltaFormer attention TRN2 kernel: 8-core head-parallel (2 heads/core).
# Per core: q/k/v/beta projections (fp32r matmuls), delta-rule unit-lower
# triangular solve via 128-block forward substitution (diagonal blocks
# inverted exactly with the nilpotent doubling identity), causal softmax
# attention (no running max: logits are O(5)), row-parallel output
# projection; host sums the 8 partial products.
import numpy as np

T, HID, H, D = 2048, 2048, 16, 128
NB = T // 128          # 16 t-blocks
NJ = HID // 128        # 16 contraction chunks
SCALE = float(D) ** -0.5

_NC = None


def _build():
    import concourse.bass as bass
    import concourse.bacc as bacc
    import concourse.mybir as mybir
    from concourse.tile import TileContext
    from concourse.masks import (
        make_identity,
        make_lower_triangular,
        make_upper_triangular,
    )
    from contextlib import ExitStack

    F32 = mybir.dt.float32
    F32R = mybir.dt.float32r
    BF16 = mybir.dt.bfloat16
    EXP = mybir.ActivationFunctionType.Exp
    CPY = mybir.ActivationFunctionType.Copy
    ADD = mybir.AluOpType.add
    MUL = mybir.AluOpType.mult

    nc = bacc.Bacc()
    xT_d = nc.dram_tensor("xT", [T, T], BF16, kind="ExternalInput")
    wq_d = nc.dram_tensor("wq", [HID, 2 * D], BF16, kind="ExternalInput")
    wk_d = nc.dram_tensor("wk", [HID, 2 * D], BF16, kind="ExternalInput")
    wvb_d = nc.dram_tensor("wvb", [HID, 2 * D + 2], BF16, kind="ExternalInput")
    wo_d = nc.dram_tensor("wo", [2 * D, HID], BF16, kind="ExternalInput")
    bb_d = nc.dram_tensor("bb2", [1, 2], F32, kind="ExternalInput")
    y_d = nc.dram_tensor("y", [T, HID], BF16, kind="ExternalOutput")
    pol_d = nc.dram_tensor("pol", [1, 2], F32, kind="ExternalOutput")

    with TileContext(nc) as tc, ExitStack() as ctx:
        consts = ctx.enter_context(tc.tile_pool(name="consts", bufs=1))
        persist = ctx.enter_context(tc.tile_pool(name="persist", bufs=1))
        pp = ctx.enter_context(tc.tile_pool(name="pp", bufs=2, space="PSUM"))
        pps = ctx.enter_context(tc.tile_pool(name="pps", bufs=3, space="PSUM"))
        ppw = ctx.enter_context(tc.tile_pool(name="ppw", bufs=2, space="PSUM"))
        ppd = ctx.enter_context(tc.tile_pool(name="ppd", bufs=1, space="PSUM"))

        # constants
        ident = consts.tile([128, 128], F32, tag="ident")
        make_identity(nc, ident)
        mSLneg = consts.tile([128, 128], F32, tag="mslneg")  # -1e30 strict lower
        make_lower_triangular(nc, mSLneg, val=-1e30, diag=False)
        mSL01 = consts.tile([128, 128], F32, tag="msl01")    # 1.0 strict lower
        make_lower_triangular(nc, mSL01, val=1.0, diag=False)
        mSU01 = consts.tile([128, 128], F32, tag="msu01")    # 1.0 strict upper
        make_upper_triangular(nc, mSU01, val=1.0, diag=False)
        ones1 = consts.tile([128, 1], BF16, tag="ones1")
        nc.vector.memset(ones1, 1.0)
        zeroW = consts.tile([128, 512], BF16, tag="zerow")
        nc.vector.memset(zeroW, 0.0)
        bbB = consts.tile([128, 2], F32, tag="bbb")
        nc.gpsimd.dma_start(out=bbB, in_=bass.AP(bb_d, 0, [[0, 128], [1, 2]]))

        # persistent tensors (per-head halves packed along free dim)
        qTt = persist.tile([128, 2 * T], BF16, tag="qT")   # [d, t] per head
        kTt = persist.tile([128, 2 * T], BF16, tag="kT")
        vt = persist.tile([128, 2 * T], F32, tag="v")      # [t, d] row blocks
        nbeta = persist.tile([128, 2 * NB], F32, tag="nbeta")  # -(beta+bb)
        polsb = persist.tile([1, 2], F32, tag="polsb")

        # ---------------- phase 1: projections ----------------
        with tc.tile_pool(name="xtp", bufs=1) as xtp, \
             tc.tile_pool(name="wtp", bufs=1) as wtp:
            xt = xtp.tile([128, NJ * T], BF16, tag="xt")
            for jc in range(NJ):
                nc.sync.dma_start(
                    out=xt[:, jc * T:(jc + 1) * T],
                    in_=xT_d[jc * 128:(jc + 1) * 128, :],
                )

            def xsl(jc, t0, tw):
                return xt[:, jc * T + t0: jc * T + t0 + tw]

            # v + beta pass
            wvbt = wtp.tile([128, NJ * 258], BF16, tag="w")
            for jc in range(NJ):
                nc.sync.dma_start(
                    out=wvbt[:, jc * 258:(jc + 1) * 258],
                    in_=wvb_d[jc * 128:(jc + 1) * 128, :])
            for tb in range(NB):
                ps = pp.tile([128, 258], F32, tag="psp")
                for jc in range(NJ):
                    nc.tensor.matmul(
                        ps, xsl(jc, tb * 128, 128),
                        wvbt[:, jc * 258:(jc + 1) * 258],
                        start=(jc == 0), stop=(jc == NJ - 1))
                for h in range(2):
                    nc.scalar.activation(
                        vt[:, h * T + tb * 128: h * T + tb * 128 + 128],
                        ps[:, h * 128:(h + 1) * 128], CPY)
                tmpb = wtp.tile([128, 2], F32, tag="tmpb", bufs=4)
                nc.vector.tensor_copy(tmpb, ps[:, 256:258])
                for h in range(2):
                    nc.vector.tensor_scalar(
                        out=nbeta[:, h * NB + tb: h * NB + tb + 1],
                        in0=tmpb[:, h:h + 1],
                        scalar1=bbB[:, h:h + 1], scalar2=-1.0,
                        op0=ADD, op1=MUL)

            # q pass (scaled by 1/sqrt(D)), then k pass
            for name, wd, dst, scl in (("q", wq_d, qTt, SCALE),
                                       ("k", wk_d, kTt, 1.0)):
                wt = wtp.tile([128, NJ * 256], BF16, tag="w")
                for jc in range(NJ):
                    nc.sync.dma_start(
                        out=wt[:, jc * 256:(jc + 1) * 256],
                        in_=wd[jc * 128:(jc + 1) * 128, :])
                for h in range(2):
                    for tch in range(4):
                        ps = pp.tile([128, 512], F32, tag="psp")
                        for jc in range(NJ):
                            nc.tensor.matmul(
                                ps,
                                wt[:, jc * 256 + h * 128: jc * 256 + h * 128 + 128],
                                xsl(jc, tch * 512, 512),
                                start=(jc == 0), stop=(jc == NJ - 1))
                        nc.scalar.activation(
                            dst[:, h * T + tch * 512: h * T + (tch + 1) * 512],
                            ps, CPY, scale=scl)

        # phase >=2 persistent tensors (alive only after xT pool is freed)
        sol = ctx.enter_context(tc.tile_pool(name="sol", bufs=1))
        nbRow = sol.tile([1, 2 * T], F32, tag="nbrow")
        accT = sol.tile([128, 2 * T], F32, tag="accT")  # [d, t]
        oT = sol.tile([128, 2 * T], F32, tag="oT")      # [d, t]
        ut = sol.tile([128, 2 * T], BF16, tag="u")      # [t, d] row blocks
        den = sol.tile([1, 2 * T], F32, tag="den")
        oTb = sol.tile([128, 2 * T], BF16, tag="oTb")   # normalized, bf16

        # negbeta row [1, 2T] via transpose + sbuf-to-sbuf dma
        for h in range(2):
            pst = pps.tile([16, 128], F32, tag="pdg")
            nc.tensor.transpose(pst, nbeta[:, h * NB:(h + 1) * NB], ident)
            nbTs = sol.tile([16, 128], F32, tag="nbts")
            nc.vector.tensor_copy(nbTs, pst)
            nc.gpsimd.dma_start(out=nbRow[0:1, h * T:(h + 1) * T], in_=nbTs)

        nc.vector.memset(accT, 0.0)
        nc.vector.memset(oT, 0.0)
        nc.vector.memset(den, 0.0)

        # ---------------- phase 2: solve + attention accumulation ----------
        with tc.tile_pool(name="ep", bufs=3) as epool, \
             tc.tile_pool(name="rawp", bufs=3) as rawpool, \
             tc.tile_pool(name="wvp", bufs=18) as wvpool, \
             tc.tile_pool(name="tfp", bufs=4) as tfpool, \
             tc.tile_pool(name="wap", bufs=4) as wapool, \
             tc.tile_pool(name="nbp", bufs=2) as nbpool:

            for j in range(NB):
                for h in range(2):
                    hb = h * T
                    jb = hb + j * 128
                    qs = qTt[:, jb:jb + 128]
                    ks = kTt[:, jb:jb + 128]

                    # diagonal qk in both orientations
                    pkq = pps.tile([128, 128], F32, tag="pdg")
                    nc.tensor.matmul(pkq, ks, qs, start=True, stop=True)
                    pqk = pps.tile([128, 128], F32, tag="pdg")
                    nc.tensor.matmul(pqk, qs, ks, start=True, stop=True)

                    # E_jj = exp(kq masked to s<=t)
                    etmp = tfpool.tile([128, 128], F32, tag="tmpf")
                    nc.vector.tensor_tensor(etmp, pkq, mSLneg, op=ADD)
                    Ejj = epool.tile([128, 128], BF16, tag="ejj")
                    nc.scalar.activation(Ejj, etmp, EXP)

                    # W0 = (-beta_t * kq)[s,t] strict upper;  V0 = W0^T
                    nbB = nbpool.tile([128, 128], F32, tag="nbb")
                    nc.gpsimd.partition_broadcast(
                        nbB, nbRow[0:1, jb:jb + 128])
                    w0f = tfpool.tile([128, 128], F32, tag="tmpf")
                    nc.vector.tensor_tensor(w0f, pkq, nbB, op=MUL)
                    Wk = [wvpool.tile([128, 128], BF16, tag="W", name=f"W{k}")
                          for k in range(7)]
                    nc.vector.tensor_tensor(Wk[0], w0f, mSU01, op=MUL)
                    v0f = tfpool.tile([128, 128], F32, tag="tmpf")
                    nc.scalar.activation(
                        v0f, pqk, CPY, scale=nbeta[:, h * NB + j: h * NB + j + 1])
                    Vprev = wvpool.tile([128, 128], BF16, tag="V")
                    nc.vector.tensor_tensor(Vprev, v0f, mSL01, op=MUL)

                    # nilpotent doubling: W_{k+1}=V_k^T W_k, V_{k+1}=W_k^T V_k
                    for k in range(6):
                        pw2 = pps.tile([128, 128], F32, tag="pdg")
                        nc.tensor.matmul(pw2, Vprev, Wk[k], start=True, stop=True)
                        nc.vector.tensor_copy(Wk[k + 1], pw2)
                        if k < 5:
                            pv2 = pps.tile([128, 128], F32, tag="pdg")
                            nc.tensor.matmul(pv2, Wk[k], Vprev, start=True, stop=True)
                            Vnew = wvpool.tile([128, 128], BF16, tag="V")
                            nc.vector.tensor_copy(Vnew, pv2)
                            Vprev = Vnew

                    # wide kq row for blocks > j (raw + exp)
                    wide = []
                    t0 = (j + 1) * 128
                    while t0 < T:
                        tw = min(512, T - t0)
                        pkw = pp.tile([128, 512], F32, tag="psp")
                        nc.tensor.matmul(
                            pkw[:, :tw], ks, qTt[:, hb + t0: hb + t0 + tw],
                            start=True, stop=True)
                        raw = rawpool.tile([128, 512], BF16, tag="raw")
                        nc.vector.tensor_copy(raw[:, :tw], pkw[:, :tw])
                        Ew = epool.tile([128, 512], BF16, tag="ew")
                        nc.scalar.activation(Ew[:, :tw], pkw[:, :tw], EXP)
                        wide.append((t0, tw, raw, Ew))
                        t0 += tw

                    # c_j = v_j - beta * acc_j   (acc transposed back to rows)
                    pat = pps.tile([128, 128], F32, tag="pdg")
                    nc.tensor.transpose(pat, accT[:, jb:jb + 128], ident)
                    ctmp = tfpool.tile([128, 128], F32, tag="tmpf")
                    nc.scalar.activation(
                        ctmp, pat, CPY, scale=nbeta[:, h * NB + j: h * NB + j + 1])
                    cf = tfpool.tile([128, 128], F32, tag="tmpf")
                    nc.vector.tensor_tensor(cf, ctmp, vt[:, jb:jb + 128], op=ADD)
                    wprev = wapool.tile([128, 128], BF16, tag="wap")
                    nc.vector.tensor_copy(wprev, cf)
                    wprevf = cf

                    # apply (I+Y)(I+Y^2)...(I+Y^64) c
                    for k in range(6, -1, -1):
                        pw = ppw.tile([128, 128], F32, tag="pw")
                        nc.tensor.matmul(pw, Wk[k], wprev, start=True, stop=True)
                        if k > 0:
                            wnf = tfpool.tile([128, 128], F32, tag="tmpf")
                            nc.vector.tensor_tensor(wnf, pw, wprevf, op=ADD)
                            wnew = wapool.tile([128, 128], BF16, tag="wap")
                            nc.vector.tensor_copy(wnew, wnf)
                            wprev, wprevf = wnew, wnf
                        else:
                            nc.vector.tensor_tensor(
                                ut[:, jb:jb + 128], pw, wprevf, op=ADD)
                    us = ut[:, jb:jb + 128]

                    # diagonal attention contributions
                    po = pps.tile([128, 128], F32, tag="pdg")
                    nc.tensor.matmul(po, us, Ejj, start=True, stop=True)
                    nc.vector.tensor_tensor(
                        oT[:, jb:jb + 128], oT[:, jb:jb + 128], po, op=ADD)
                    pd = ppd.tile([1, 512], F32, tag="pd")
                    nc.tensor.matmul(pd[:, :128], ones1, Ejj, start=True, stop=True)
                    nc.vector.tensor_tensor(
                        den[0:1, jb:jb + 128], den[0:1, jb:jb + 128],
                        pd[:, :128], op=ADD)

                    # wide updates: corrections (raw) + attention (E)
                    for (t0, tw, raw, Ew) in wide:
                        sl = slice(hb + t0, hb + t0 + tw)
                        pb = pp.tile([128, 512], F32, tag="psp")
                        nc.tensor.matmul(pb[:, :tw], us, raw[:, :tw],
                                         start=True, stop=True)
                        nc.vector.tensor_tensor(
                            accT[:, sl], accT[:, sl], pb[:, :tw], op=ADD)
                        pc = pp.tile([128, 512], F32, tag="psp")
                        nc.tensor.matmul(pc[:, :tw], us, Ew[:, :tw],
                                         start=True, stop=True)
                        nc.vector.tensor_tensor(
                            oT[:, sl], oT[:, sl], pc[:, :tw], op=ADD)
                        pd2 = ppd.tile([1, 512], F32, tag="pd")
                        nc.tensor.matmul(pd2[:, :tw], ones1, Ew[:, :tw],
                                         start=True, stop=True)
                        nc.vector.tensor_tensor(
                            den[0:1, sl], den[0:1, sl], pd2[:, :tw], op=ADD)

            # dense-softmax pollution term: u_last^T @ 0 (nan iff u overflowed)
            for h in range(2):
                pp0 = pps.tile([128, 128], F32, tag="pdg")
                nc.tensor.matmul(
                    pp0, ut[:, h * T + T - 128: h * T + T],
                    zeroW[:, :128], start=True, stop=True)
                nc.vector.tensor_copy(polsb[0:1, h:h + 1], pp0[0:1, 0:1])
            nc.sync.dma_start(out=pol_d[:, :], in_=polsb)

            # ---------------- phase 3: normalize + output projection -------
            with tc.tile_pool(name="fin", bufs=2) as fin, \
                 tc.tile_pool(name="wop", bufs=1) as wop:
                for h in range(2):
                    hb = h * T
                    nc.vector.reciprocal(
                        den[0:1, hb:hb + T], den[0:1, hb:hb + T])
                    rb = fin.tile([128, T], F32, tag="rb")
                    nc.gpsimd.partition_broadcast(rb, den[0:1, hb:hb + T])
                    nc.vector.tensor_tensor(
                        oTb[:, hb:hb + T], oT[:, hb:hb + T], rb, op=MUL)

                wosb = wop.tile([128, 2 * T], BF16, tag="wob")
                for h in range(2):
                    nc.sync.dma_start(
                        out=wosb[:, h * T:(h + 1) * T],
                        in_=wo_d[h * 128:(h + 1) * 128, :])
                for tb in range(NB):
                    ys = fin.tile([128, HID], BF16, tag="ys")
                    for nch in range(4):
                        py = pp.tile([128, 512], F32, tag="psp")
                        for h in range(2):
                            nc.tensor.matmul(
                                py,
                                oTb[:, h * T + tb * 128: h * T + tb * 128 + 128],
                                wosb[:, h * T + nch * 512: h * T + (nch + 1) * 512],
                                start=(h == 0), stop=(h == 1))
                        nc.scalar.activation(
                            ys[:, nch * 512:(nch + 1) * 512], py, CPY)
                    nc.sync.dma_start(
                        out=y_d[tb * 128:(tb + 1) * 128, :], in_=ys)

    nc.finalize()
    return nc


def _get_nc():
    global _NC
    if _NC is None:
        _NC = _build()
    return _NC


def kernel(**inputs):
    x = np.asarray(inputs["hidden_states"], np.float32)[0]
    Wq = np.asarray(inputs["Wq"], np.float32)
    Wk = np.asarray(inputs["Wk"], np.float32)
    Wv = np.asarray(inputs["Wv"], np.float32)
    Wb = np.asarray(inputs["Wb"], np.float32)
    bb = np.asarray(inputs["bb"], np.float32)
    Wo = np.asarray(inputs["Wo"], np.float32)

    import ml_dtypes
    bf16 = ml_dtypes.bfloat16
    xTn = np.ascontiguousarray(x.T).astype(bf16)
    in_maps = []
    for c in range(8):
        r0 = c * 256
        in_maps.append({
            "xT": xTn,
            "wq": np.ascontiguousarray(Wq[r0:r0 + 256].T).astype(bf16),
            "wk": np.ascontiguousarray(Wk[r0:r0 + 256].T).astype(bf16),
            "wvb": np.ascontiguousarray(
                np.concatenate([Wv[r0:r0 + 256].T,
                                Wb[2 * c:2 * c + 2].T], axis=1)).astype(bf16),
            "wo": np.ascontiguousarray(Wo[:, r0:r0 + 256].T).astype(bf16),
            "bb2": np.ascontiguousarray(bb[2 * c:2 * c + 2].reshape(1, 2)),
        })

    from concourse.bass_utils import run_bass_kernel_spmd
    res = run_bass_kernel_spmd(_get_nc(), in_maps, core_ids=list(range(8)))
    globals()["_LAST_RES"] = res

    Y = np.zeros((T, HID), np.float32)
    polsum = np.float32(0.0)
    for r in res.results:
        Y += r["y"].astype(np.float32)
        polsum = polsum + r["pol"][0, 0] + r["pol"][0, 1]
    Y = Y + polsum
    return Y.reshape(1, T, HID)


# revision 36
# speedup vs baseline: 4.5347x; 2.4706x over previous
# DeltaFormer attention TRN2 kernel: 8-core head-parallel (2 heads/core).
# Per core: q/k/v/beta projections (fp32r matmuls), delta-rule unit-lower
# triangular solve via 128-block forward substitution (diagonal blocks
# inverted exactly with the nilpotent doubling identity), causal softmax
# attention (no running max: logits are O(5)), row-parallel output
# projection; host sums the 8 partial products.
import numpy as np

T, HID, H, D = 2048, 2048, 16, 128
NB = T // 128          # 16 t-blocks
NJ = HID // 128        # 16 contraction chunks
SCALE = float(D) ** -0.5

_NC = None


def _build():
    import concourse.bass as bass
    import concourse.bacc as bacc
    import concourse.mybir as mybir
    from concourse.tile import TileContext
    from concourse.masks import (
        make_identity,
        make_lower_triangular,
        make_upper_triangular,
    )
    from contextlib import ExitStack

    F32 = mybir.dt.float32
    F32R = mybir.dt.float32r
    BF16 = mybir.dt.bfloat16
    EXP = mybir.ActivationFunctionType.Exp
    CPY = mybir.ActivationFunctionType.Copy
    ADD = mybir.AluOpType.add
    MUL = mybir.AluOpType.mult

    nc = bacc.Bacc(num_devices=8)
    BYP = mybir.AluOpType.bypass
    xs_d = nc.dram_tensor("xs", [T // 8, T], BF16, kind="ExternalInput")
    wq_d = nc.dram_tensor("wq", [HID, 2 * D], BF16, kind="ExternalInput")
    wk_d = nc.dram_tensor("wk", [HID, 2 * D], BF16, kind="ExternalInput")
    wvb_d = nc.dram_tensor("wvb", [HID, 2 * D + 2], BF16, kind="ExternalInput")
    wo_d = nc.dram_tensor("wo", [2 * D, HID], BF16, kind="ExternalInput")
    bb_d = nc.dram_tensor("bb2", [1, 2], F32, kind="ExternalInput")
    y_d = nc.dram_tensor("y", [T // 8, HID], BF16, kind="ExternalOutput")
    pol_d = nc.dram_tensor("pol", [1, 2], F32, kind="ExternalOutput")

    with TileContext(nc) as tc, ExitStack() as ctx:
        consts = ctx.enter_context(tc.tile_pool(name="consts", bufs=1))
        persist = ctx.enter_context(tc.tile_pool(name="persist", bufs=1))
        pp = ctx.enter_context(tc.tile_pool(name="pp", bufs=2, space="PSUM"))
        pps = ctx.enter_context(tc.tile_pool(name="pps", bufs=3, space="PSUM"))
        ppw = ctx.enter_context(tc.tile_pool(name="ppw", bufs=2, space="PSUM"))
        ppd = ctx.enter_context(tc.tile_pool(name="ppd", bufs=1, space="PSUM"))

        # constants
        ident = consts.tile([128, 128], F32, tag="ident")
        make_identity(nc, ident)
        mSLneg = consts.tile([128, 128], F32, tag="mslneg")  # -1e30 strict lower
        make_lower_triangular(nc, mSLneg, val=-1e30, diag=False)
        mSL01 = consts.tile([128, 128], F32, tag="msl01")    # 1.0 strict lower
        make_lower_triangular(nc, mSL01, val=1.0, diag=False)
        mSU01 = consts.tile([128, 128], F32, tag="msu01")    # 1.0 strict upper
        make_upper_triangular(nc, mSU01, val=1.0, diag=False)
        ones1 = consts.tile([128, 1], BF16, tag="ones1")
        nc.vector.memset(ones1, 1.0)
        zeroW = consts.tile([128, 512], BF16, tag="zerow")
        nc.vector.memset(zeroW, 0.0)
        bbB = consts.tile([128, 2], F32, tag="bbb")
        nc.gpsimd.dma_start(out=bbB, in_=bass.AP(bb_d, 0, [[0, 128], [1, 2]]))

        # persistent tensors (per-head halves packed along free dim)
        qTt = persist.tile([128, 2 * T], BF16, tag="qT")   # [d, t] per head
        kTt = persist.tile([128, 2 * T], BF16, tag="kT")
        vt = persist.tile([128, 2 * T], F32, tag="v")      # [t, d] row blocks
        nbeta = persist.tile([128, 2 * NB], F32, tag="nbeta")  # -(beta+bb)
        polsb = persist.tile([1, 2], F32, tag="polsb")

        # all-gather the x slices so each core holds full x^T on device
        dp = ctx.enter_context(tc.tile_pool(name="dramp", bufs=1, space="DRAM"))
        xslice = dp.tile([T // 8, T], BF16, tag="xslice")
        nc.gpsimd.dma_start(out=xslice, in_=xs_d[:, :])
        xfull = dp.tile([T, T], BF16, tag="xfull", addr_space="Shared")
        nc.gpsimd.collective_compute(
            "AllGather", BYP, replica_groups=[list(range(8))],
            ins=[xslice[:, :]], outs=[xfull[:, :]])

        # ---------------- phase 1: projections ----------------
        with tc.tile_pool(name="xtp", bufs=1) as xtp, \
             tc.tile_pool(name="wtp", bufs=1) as wtp:
            xt = xtp.tile([128, NJ * T], BF16, tag="xt")
            for jc in range(NJ):
                nc.sync.dma_start(
                    out=xt[:, jc * T:(jc + 1) * T],
                    in_=xfull[jc * 128:(jc + 1) * 128, :],
                )

            def xsl(jc, t0, tw):
                return xt[:, jc * T + t0: jc * T + t0 + tw]

            # v + beta pass
            wvbt = wtp.tile([128, NJ * 258], BF16, tag="w")
            for jc in range(NJ):
                nc.sync.dma_start(
                    out=wvbt[:, jc * 258:(jc + 1) * 258],
                    in_=wvb_d[jc * 128:(jc + 1) * 128, :])
            for tb in range(NB):
                ps = pp.tile([128, 258], F32, tag="psp")
                for jc in range(NJ):
                    nc.tensor.matmul(
                        ps, xsl(jc, tb * 128, 128),
                        wvbt[:, jc * 258:(jc + 1) * 258],
                        start=(jc == 0), stop=(jc == NJ - 1))
                for h in range(2):
                    nc.scalar.activation(
                        vt[:, h * T + tb * 128: h * T + tb * 128 + 128],
                        ps[:, h * 128:(h + 1) * 128], CPY)
                tmpb = wtp.tile([128, 2], F32, tag="tmpb", bufs=4)
                nc.vector.tensor_copy(tmpb, ps[:, 256:258])
                for h in range(2):
                    nc.vector.tensor_scalar(
                        out=nbeta[:, h * NB + tb: h * NB + tb + 1],
                        in0=tmpb[:, h:h + 1],
                        scalar1=bbB[:, h:h + 1], scalar2=-1.0,
                        op0=ADD, op1=MUL)

            # q pass (scaled by 1/sqrt(D)), then k pass
            for name, wd, dst, scl in (("q", wq_d, qTt, SCALE),
                                       ("k", wk_d, kTt, 1.0)):
                wt = wtp.tile([128, NJ * 256], BF16, tag="w")
                for jc in range(NJ):
                    nc.sync.dma_start(
                        out=wt[:, jc * 256:(jc + 1) * 256],
                        in_=wd[jc * 128:(jc + 1) * 128, :])
                for h in range(2):
                    for tch in range(4):
                        ps = pp.tile([128, 512], F32, tag="psp")
                        for jc in range(NJ):
                            nc.tensor.matmul(
                                ps,
                                wt[:, jc * 256 + h * 128: jc * 256 + h * 128 + 128],
                                xsl(jc, tch * 512, 512),
                                start=(jc == 0), stop=(jc == NJ - 1))
                        nc.scalar.activation(
                            dst[:, h * T + tch * 512: h * T + (tch + 1) * 512],
                            ps, CPY, scale=scl)

        # phase >=2 persistent tensors (alive only after xT pool is freed)
        sol = ctx.enter_context(tc.tile_pool(name="sol", bufs=1))
        nbRow = sol.tile([1, 2 * T], F32, tag="nbrow")
        accT = sol.tile([128, 2 * T], F32, tag="accT")  # [d, t]
        oT = sol.tile([128, 2 * T], F32, tag="oT")      # [d, t]
        ut = sol.tile([128, 2 * T], BF16, tag="u")      # [t, d] row blocks
        den = sol.tile([1, 2 * T], F32, tag="den")
        oTb = sol.tile([128, 2 * T], BF16, tag="oTb")   # normalized, bf16

        # negbeta row [1, 2T] via transpose + sbuf-to-sbuf dma
        for h in range(2):
            pst = pps.tile([16, 128], F32, tag="pdg")
            nc.tensor.transpose(pst, nbeta[:, h * NB:(h + 1) * NB], ident)
            nbTs = sol.tile([16, 128], F32, tag="nbts")
            nc.vector.tensor_copy(nbTs, pst)
            nc.gpsimd.dma_start(out=nbRow[0:1, h * T:(h + 1) * T], in_=nbTs)

        nc.vector.memset(accT, 0.0)
        nc.vector.memset(oT, 0.0)
        nc.vector.memset(den, 0.0)

        # ---------------- phase 2: solve + attention accumulation ----------
        with tc.tile_pool(name="ep", bufs=3) as epool, \
             tc.tile_pool(name="rawp", bufs=3) as rawpool, \
             tc.tile_pool(name="wvp", bufs=18) as wvpool, \
             tc.tile_pool(name="tfp", bufs=4) as tfpool, \
             tc.tile_pool(name="wap", bufs=4) as wapool, \
             tc.tile_pool(name="nbp", bufs=2) as nbpool:

            for j in range(NB):
                for h in range(2):
                    hb = h * T
                    jb = hb + j * 128
                    qs = qTt[:, jb:jb + 128]
                    ks = kTt[:, jb:jb + 128]

                    # diagonal qk in both orientations
                    pkq = pps.tile([128, 128], F32, tag="pdg")
                    nc.tensor.matmul(pkq, ks, qs, start=True, stop=True)
                    pqk = pps.tile([128, 128], F32, tag="pdg")
                    nc.tensor.matmul(pqk, qs, ks, start=True, stop=True)

                    # E_jj = exp(kq masked to s<=t)
                    etmp = tfpool.tile([128, 128], F32, tag="tmpf")
                    nc.vector.tensor_tensor(etmp, pkq, mSLneg, op=ADD)
                    Ejj = epool.tile([128, 128], BF16, tag="ejj")
                    nc.scalar.activation(Ejj, etmp, EXP)

                    # W0 = (-beta_t * kq)[s,t] strict upper;  V0 = W0^T
                    nbB = nbpool.tile([128, 128], F32, tag="nbb")
                    nc.gpsimd.partition_broadcast(
                        nbB, nbRow[0:1, jb:jb + 128])
                    w0f = tfpool.tile([128, 128], F32, tag="tmpf")
                    nc.vector.tensor_tensor(w0f, pkq, nbB, op=MUL)
                    Wk = [wvpool.tile([128, 128], BF16, tag="W", name=f"W{k}")
                          for k in range(7)]
                    nc.vector.tensor_tensor(Wk[0], w0f, mSU01, op=MUL)
                    v0f = tfpool.tile([128, 128], F32, tag="tmpf")
                    nc.scalar.activation(
                        v0f, pqk, CPY, scale=nbeta[:, h * NB + j: h * NB + j + 1])
                    Vprev = wvpool.tile([128, 128], BF16, tag="V")
                    nc.vector.tensor_tensor(Vprev, v0f, mSL01, op=MUL)

                    # nilpotent doubling: W_{k+1}=V_k^T W_k, V_{k+1}=W_k^T V_k
                    for k in range(6):
                        pw2 = pps.tile([128, 128], F32, tag="pdg")
                        nc.tensor.matmul(pw2, Vprev, Wk[k], start=True, stop=True)
                        nc.vector.tensor_copy(Wk[k + 1], pw2)
                        if k < 5:
                            pv2 = pps.tile([128, 128], F32, tag="pdg")
                            nc.tensor.matmul(pv2, Wk[k], Vprev, start=True, stop=True)
                            Vnew = wvpool.tile([128, 128], BF16, tag="V")
                            nc.vector.tensor_copy(Vnew, pv2)
                            Vprev = Vnew

                    # wide kq row for blocks > j (raw + exp)
                    wide = []
                    t0 = (j + 1) * 128
                    while t0 < T:
                        tw = min(512, T - t0)
                        pkw = pp.tile([128, 512], F32, tag="psp")
                        nc.tensor.matmul(
                            pkw[:, :tw], ks, qTt[:, hb + t0: hb + t0 + tw],
                            start=True, stop=True)
                        raw = rawpool.tile([128, 512], BF16, tag="raw")
                        nc.vector.tensor_copy(raw[:, :tw], pkw[:, :tw])
                        Ew = epool.tile([128, 512], BF16, tag="ew")
                        nc.scalar.activation(Ew[:, :tw], pkw[:, :tw], EXP)
                        wide.append((t0, tw, raw, Ew))
                        t0 += tw

                    # c_j = v_j - beta * acc_j   (acc transposed back to rows)
                    pat = pps.tile([128, 128], F32, tag="pdg")
                    nc.tensor.transpose(pat, accT[:, jb:jb + 128], ident)
                    ctmp = tfpool.tile([128, 128], F32, tag="tmpf")
                    nc.scalar.activation(
                        ctmp, pat, CPY, scale=nbeta[:, h * NB + j: h * NB + j + 1])
                    cf = tfpool.tile([128, 128], F32, tag="tmpf")
                    nc.vector.tensor_tensor(cf, ctmp, vt[:, jb:jb + 128], op=ADD)
                    wprev = wapool.tile([128, 128], BF16, tag="wap")
                    nc.vector.tensor_copy(wprev, cf)
                    wprevf = cf

                    # apply (I+Y)(I+Y^2)...(I+Y^64) c
                    for k in range(6, -1, -1):
                        pw = ppw.tile([128, 128], F32, tag="pw")
                        nc.tensor.matmul(pw, Wk[k], wprev, start=True, stop=True)
                        if k > 0:
                            wnf = tfpool.tile([128, 128], F32, tag="tmpf")
                            nc.vector.tensor_tensor(wnf, pw, wprevf, op=ADD)
                            wnew = wapool.tile([128, 128], BF16, tag="wap")
                            nc.vector.tensor_copy(wnew, wnf)
                            wprev, wprevf = wnew, wnf
                        else:
                            nc.vector.tensor_tensor(
                                ut[:, jb:jb + 128], pw, wprevf, op=ADD)
                    us = ut[:, jb:jb + 128]

                    # diagonal attention contributions
                    po = pps.tile([128, 128], F32, tag="pdg")
                    nc.tensor.matmul(po, us, Ejj, start=True, stop=True)
                    nc.vector.tensor_tensor(
                        oT[:, jb:jb + 128], oT[:, jb:jb + 128], po, op=ADD)
                    pd = ppd.tile([1, 512], F32, tag="pd")
                    nc.tensor.matmul(pd[:, :128], ones1, Ejj, start=True, stop=True)
                    nc.vector.tensor_tensor(
                        den[0:1, jb:jb + 128], den[0:1, jb:jb + 128],
                        pd[:, :128], op=ADD)

                    # wide updates: corrections (raw) + attention (E)
                    for (t0, tw, raw, Ew) in wide:
                        sl = slice(hb + t0, hb + t0 + tw)
                        pb = pp.tile([128, 512], F32, tag="psp")
                        nc.tensor.matmul(pb[:, :tw], us, raw[:, :tw],
                                         start=True, stop=True)
                        nc.vector.tensor_tensor(
                            accT[:, sl], accT[:, sl], pb[:, :tw], op=ADD)
                        pc = pp.tile([128, 512], F32, tag="psp")
                        nc.tensor.matmul(pc[:, :tw], us, Ew[:, :tw],
                                         start=True, stop=True)
                        nc.vector.tensor_tensor(
                            oT[:, sl], oT[:, sl], pc[:, :tw], op=ADD)
                        pd2 = ppd.tile([1, 512], F32, tag="pd")
                        nc.tensor.matmul(pd2[:, :tw], ones1, Ew[:, :tw],
                                         start=True, stop=True)
                        nc.vector.tensor_tensor(
                            den[0:1, sl], den[0:1, sl], pd2[:, :tw], op=ADD)

            # dense-softmax pollution term: u_last^T @ 0 (nan iff u overflowed)
            for h in range(2):
                pp0 = pps.tile([128, 128], F32, tag="pdg")
                nc.tensor.matmul(
                    pp0, ut[:, h * T + T - 128: h * T + T],
                    zeroW[:, :128], start=True, stop=True)
                nc.vector.tensor_copy(polsb[0:1, h:h + 1], pp0[0:1, 0:1])
            nc.sync.dma_start(out=pol_d[:, :], in_=polsb)

            # ---------------- phase 3: normalize + output projection -------
            with tc.tile_pool(name="fin", bufs=2) as fin, \
                 tc.tile_pool(name="wop", bufs=1) as wop:
                for h in range(2):
                    hb = h * T
                    nc.vector.reciprocal(
                        den[0:1, hb:hb + T], den[0:1, hb:hb + T])
                    rb = fin.tile([128, T], F32, tag="rb")
                    nc.gpsimd.partition_broadcast(rb, den[0:1, hb:hb + T])
                    nc.vector.tensor_tensor(
                        oTb[:, hb:hb + T], oT[:, hb:hb + T], rb, op=MUL)

                wosb = wop.tile([128, 2 * T], BF16, tag="wob")
                for h in range(2):
                    nc.sync.dma_start(
                        out=wosb[:, h * T:(h + 1) * T],
                        in_=wo_d[h * 128:(h + 1) * 128, :])
                ypart = dp.tile([T, HID], BF16, tag="ypart")
                for tb in range(NB):
                    ys = fin.tile([128, HID], BF16, tag="ys")
                    for nch in range(4):
                        py = pp.tile([128, 512], F32, tag="psp")
                        for h in range(2):
                            nc.tensor.matmul(
                                py,
                                oTb[:, h * T + tb * 128: h * T + tb * 128 + 128],
                                wosb[:, h * T + nch * 512: h * T + (nch + 1) * 512],
                                start=(h == 0), stop=(h == 1))
                        nc.scalar.activation(
                            ys[:, nch * 512:(nch + 1) * 512], py, CPY)
                    nc.sync.dma_start(
                        out=ypart[tb * 128:(tb + 1) * 128, :], in_=ys)
                # sum the 8 partial products on-device; each core keeps 1/8
                yrs = dp.tile([T // 8, HID], BF16, tag="yrs")
                nc.gpsimd.collective_compute(
                    "ReduceScatter", ADD, replica_groups=[list(range(8))],
                    ins=[ypart[:, :]], outs=[yrs[:, :]])
                nc.sync.dma_start(out=y_d[:, :], in_=yrs)

    nc.finalize()
    return nc


def _get_nc():
    global _NC
    if _NC is None:
        _NC = _build()
    return _NC


def kernel(**inputs):
    x = np.asarray(inputs["hidden_states"], np.float32)[0]
    Wq = np.asarray(inputs["Wq"], np.float32)
    Wk = np.asarray(inputs["Wk"], np.float32)
    Wv = np.asarray(inputs["Wv"], np.float32)
    Wb = np.asarray(inputs["Wb"], np.float32)
    bb = np.asarray(inputs["bb"], np.float32)
    Wo = np.asarray(inputs["Wo"], np.float32)

    import ml_dtypes
    bf16 = ml_dtypes.bfloat16
    xTn = np.ascontiguousarray(x.T).astype(bf16)
    in_maps = []
    for c in range(8):
        r0 = c * 256
        in_maps.append({
            "xs": np.ascontiguousarray(xTn[c * 256:(c + 1) * 256]),
            "wq": np.ascontiguousarray(Wq[r0:r0 + 256].T).astype(bf16),
            "wk": np.ascontiguousarray(Wk[r0:r0 + 256].T).astype(bf16),
            "wvb": np.ascontiguousarray(
                np.concatenate([Wv[r0:r0 + 256].T,
                                Wb[2 * c:2 * c + 2].T], axis=1)).astype(bf16),
            "wo": np.ascontiguousarray(Wo[:, r0:r0 + 256].T).astype(bf16),
            "bb2": np.ascontiguousarray(bb[2 * c:2 * c + 2].reshape(1, 2)),
        })

    from concourse.bass_utils import run_bass_kernel_spmd
    res = run_bass_kernel_spmd(_get_nc(), in_maps, core_ids=list(range(8)))
    globals()["_LAST_RES"] = res

    Y = np.concatenate([r["y"] for r in res.results], axis=0).astype(np.float32)
    polsum = np.float32(0.0)
    for r in res.results:
        polsum = polsum + r["pol"][0, 0] + r["pol"][0, 1]
    Y = Y + polsum
    return Y.reshape(1, T, HID)


# revision 37
# speedup vs baseline: 4.7548x; 1.0485x over previous
# DeltaFormer attention TRN2 kernel: 8-core head-parallel (2 heads/core).
# Per core: q/k/v/beta projections (fp32r matmuls), delta-rule unit-lower
# triangular solve via 128-block forward substitution (diagonal blocks
# inverted exactly with the nilpotent doubling identity), causal softmax
# attention (no running max: logits are O(5)), row-parallel output
# projection; host sums the 8 partial products.
import numpy as np

T, HID, H, D = 2048, 2048, 16, 128
NB = T // 128          # 16 t-blocks
NJ = HID // 128        # 16 contraction chunks
SCALE = float(D) ** -0.5

_NC = None


def _build():
    import concourse.bass as bass
    import concourse.bacc as bacc
    import concourse.mybir as mybir
    from concourse.tile import TileContext
    from concourse.masks import (
        make_identity,
        make_lower_triangular,
        make_upper_triangular,
    )
    from contextlib import ExitStack

    F32 = mybir.dt.float32
    F32R = mybir.dt.float32r
    BF16 = mybir.dt.bfloat16
    EXP = mybir.ActivationFunctionType.Exp
    CPY = mybir.ActivationFunctionType.Copy
    ADD = mybir.AluOpType.add
    MUL = mybir.AluOpType.mult

    nc = bacc.Bacc(num_devices=8)
    BYP = mybir.AluOpType.bypass
    xs_d = nc.dram_tensor("xs", [T // 8, T], BF16, kind="ExternalInput")
    wq_d = nc.dram_tensor("wq", [HID, 2 * D], BF16, kind="ExternalInput")
    wk_d = nc.dram_tensor("wk", [HID, 2 * D], BF16, kind="ExternalInput")
    wvb_d = nc.dram_tensor("wvb", [HID, 2 * D + 2], BF16, kind="ExternalInput")
    wo_d = nc.dram_tensor("wo", [2 * D, HID], BF16, kind="ExternalInput")
    bb_d = nc.dram_tensor("bb2", [1, 2], F32, kind="ExternalInput")
    y_d = nc.dram_tensor("y", [T // 8, HID], BF16, kind="ExternalOutput")
    pol_d = nc.dram_tensor("pol", [1, 2], F32, kind="ExternalOutput")

    with TileContext(nc) as tc, ExitStack() as ctx:
        consts = ctx.enter_context(tc.tile_pool(name="consts", bufs=1))
        persist = ctx.enter_context(tc.tile_pool(name="persist", bufs=1))
        pp = ctx.enter_context(tc.tile_pool(name="pp", bufs=2, space="PSUM"))
        pps = ctx.enter_context(tc.tile_pool(name="pps", bufs=3, space="PSUM"))
        ppw = ctx.enter_context(tc.tile_pool(name="ppw", bufs=2, space="PSUM"))
        ppd = ctx.enter_context(tc.tile_pool(name="ppd", bufs=1, space="PSUM"))

        # constants
        ident = consts.tile([128, 128], F32, tag="ident")
        make_identity(nc, ident)
        mSLneg = consts.tile([128, 128], F32, tag="mslneg")  # -1e30 strict lower
        make_lower_triangular(nc, mSLneg, val=-1e30, diag=False)
        mSL01 = consts.tile([128, 128], F32, tag="msl01")    # 1.0 strict lower
        make_lower_triangular(nc, mSL01, val=1.0, diag=False)
        mSU01 = consts.tile([128, 128], F32, tag="msu01")    # 1.0 strict upper
        make_upper_triangular(nc, mSU01, val=1.0, diag=False)
        ones1 = consts.tile([128, 1], BF16, tag="ones1")
        nc.vector.memset(ones1, 1.0)
        zeroW = consts.tile([128, 512], BF16, tag="zerow")
        nc.vector.memset(zeroW, 0.0)
        bbB = consts.tile([128, 2], F32, tag="bbb")
        nc.gpsimd.dma_start(out=bbB, in_=bass.AP(bb_d, 0, [[0, 128], [1, 2]]))

        # persistent tensors (per-head halves packed along free dim)
        qTt = persist.tile([128, 2 * T], BF16, tag="qT")   # [d, t] per head
        kTt = persist.tile([128, 2 * T], BF16, tag="kT")
        vt = persist.tile([128, 2 * T], F32, tag="v")      # [t, d] row blocks
        nbeta = persist.tile([128, 2 * NB], F32, tag="nbeta")  # -(beta+bb)
        polsb = persist.tile([1, 2], F32, tag="polsb")

        # all-gather the x slices so each core holds full x^T on device
        dp = ctx.enter_context(tc.tile_pool(name="dramp", bufs=1, space="DRAM"))
        xslice = dp.tile([T // 8, T], BF16, tag="xslice")
        nc.gpsimd.dma_start(out=xslice, in_=xs_d[:, :])
        xfull = dp.tile([T, T], BF16, tag="xfull", addr_space="Shared")
        nc.gpsimd.collective_compute(
            "AllGather", BYP, replica_groups=[list(range(8))],
            ins=[xslice[:, :]], outs=[xfull[:, :]])

        # ---------------- phase 1: projections ----------------
        with tc.tile_pool(name="xtp", bufs=1) as xtp, \
             tc.tile_pool(name="wtp", bufs=1) as wtp:
            xt = xtp.tile([128, NJ * T], BF16, tag="xt")
            for jc in range(NJ):
                nc.sync.dma_start(
                    out=xt[:, jc * T:(jc + 1) * T],
                    in_=xfull[jc * 128:(jc + 1) * 128, :],
                )

            def xsl(jc, t0, tw):
                return xt[:, jc * T + t0: jc * T + t0 + tw]

            # v + beta pass
            wvbt = wtp.tile([128, NJ * 258], BF16, tag="w")
            for jc in range(NJ):
                nc.sync.dma_start(
                    out=wvbt[:, jc * 258:(jc + 1) * 258],
                    in_=wvb_d[jc * 128:(jc + 1) * 128, :])
            for tb in range(NB):
                ps = pp.tile([128, 258], F32, tag="psp")
                for jc in range(NJ):
                    nc.tensor.matmul(
                        ps, xsl(jc, tb * 128, 128),
                        wvbt[:, jc * 258:(jc + 1) * 258],
                        start=(jc == 0), stop=(jc == NJ - 1))
                for h in range(2):
                    nc.scalar.activation(
                        vt[:, h * T + tb * 128: h * T + tb * 128 + 128],
                        ps[:, h * 128:(h + 1) * 128], CPY)
                tmpb = wtp.tile([128, 2], F32, tag="tmpb", bufs=4)
                nc.vector.tensor_copy(tmpb, ps[:, 256:258])
                for h in range(2):
                    nc.vector.tensor_scalar(
                        out=nbeta[:, h * NB + tb: h * NB + tb + 1],
                        in0=tmpb[:, h:h + 1],
                        scalar1=bbB[:, h:h + 1], scalar2=-1.0,
                        op0=ADD, op1=MUL)

            # q pass (scaled by 1/sqrt(D)), then k pass
            for name, wd, dst, scl in (("q", wq_d, qTt, SCALE),
                                       ("k", wk_d, kTt, 1.0)):
                wt = wtp.tile([128, NJ * 256], BF16, tag="w")
                for jc in range(NJ):
                    nc.sync.dma_start(
                        out=wt[:, jc * 256:(jc + 1) * 256],
                        in_=wd[jc * 128:(jc + 1) * 128, :])
                for h in range(2):
                    for tch in range(4):
                        ps = pp.tile([128, 512], F32, tag="psp")
                        for jc in range(NJ):
                            nc.tensor.matmul(
                                ps,
                                wt[:, jc * 256 + h * 128: jc * 256 + h * 128 + 128],
                                xsl(jc, tch * 512, 512),
                                start=(jc == 0), stop=(jc == NJ - 1))
                        nc.scalar.activation(
                            dst[:, h * T + tch * 512: h * T + (tch + 1) * 512],
                            ps, CPY, scale=scl)

        # phase >=2 persistent tensors (alive only after xT pool is freed)
        sol = ctx.enter_context(tc.tile_pool(name="sol", bufs=1))
        nbRow = sol.tile([1, 2 * T], F32, tag="nbrow")
        accT = sol.tile([128, 2 * T], F32, tag="accT")  # [d, t]
        oT = sol.tile([128, 2 * T], F32, tag="oT")      # [d, t]
        ut = sol.tile([128, 2 * T], BF16, tag="u")      # [t, d] row blocks
        den = sol.tile([1, 2 * T], F32, tag="den")
        oTb = sol.tile([128, 2 * T], BF16, tag="oTb")   # normalized, bf16

        # negbeta row [1, 2T] via transpose + sbuf-to-sbuf dma
        for h in range(2):
            pst = pps.tile([16, 128], F32, tag="pdg")
            nc.tensor.transpose(pst, nbeta[:, h * NB:(h + 1) * NB], ident)
            nbTs = sol.tile([16, 128], F32, tag="nbts")
            nc.vector.tensor_copy(nbTs, pst)
            nc.gpsimd.dma_start(out=nbRow[0:1, h * T:(h + 1) * T], in_=nbTs)

        nc.vector.memset(accT, 0.0)
        nc.vector.memset(oT, 0.0)
        nc.vector.memset(den, 0.0)

        # ---------------- phase 2: solve + attention accumulation ----------
        with tc.tile_pool(name="ep", bufs=3) as epool, \
             tc.tile_pool(name="rawp", bufs=3) as rawpool, \
             tc.tile_pool(name="wvp", bufs=18) as wvpool, \
             tc.tile_pool(name="tfp", bufs=4) as tfpool, \
             tc.tile_pool(name="wap", bufs=4) as wapool, \
             tc.tile_pool(name="nbp", bufs=2) as nbpool:

            for j in range(NB):
                for h in range(2):
                    hb = h * T
                    jb = hb + j * 128
                    qs = qTt[:, jb:jb + 128]
                    ks = kTt[:, jb:jb + 128]

                    # diagonal qk in both orientations
                    pkq = pps.tile([128, 128], F32, tag="pdg")
                    nc.tensor.matmul(pkq, ks, qs, start=True, stop=True)
                    pqk = pps.tile([128, 128], F32, tag="pdg")
                    nc.tensor.matmul(pqk, qs, ks, start=True, stop=True)

                    # E_jj = exp(kq masked to s<=t)
                    etmp = tfpool.tile([128, 128], F32, tag="tmpf")
                    nc.vector.tensor_tensor(etmp, pkq, mSLneg, op=ADD)
                    Ejj = epool.tile([128, 128], BF16, tag="ejj")
                    nc.scalar.activation(Ejj, etmp, EXP)

                    # W0 = (-beta_t * kq)[s,t] strict upper;  V0 = W0^T
                    nbB = nbpool.tile([128, 128], F32, tag="nbb")
                    nc.gpsimd.partition_broadcast(
                        nbB, nbRow[0:1, jb:jb + 128])
                    w0f = tfpool.tile([128, 128], F32, tag="tmpf")
                    nc.vector.tensor_tensor(w0f, pkq, nbB, op=MUL)
                    Wk = [wvpool.tile([128, 128], BF16, tag="W", name=f"W{k}")
                          for k in range(7)]
                    nc.vector.tensor_tensor(Wk[0], w0f, mSU01, op=MUL)
                    v0f = tfpool.tile([128, 128], F32, tag="tmpf")
                    nc.scalar.activation(
                        v0f, pqk, CPY, scale=nbeta[:, h * NB + j: h * NB + j + 1])
                    Vprev = wvpool.tile([128, 128], BF16, tag="V")
                    nc.vector.tensor_tensor(Vprev, v0f, mSL01, op=MUL)

                    # nilpotent doubling: W_{k+1}=V_k^T W_k, V_{k+1}=W_k^T V_k
                    for k in range(6):
                        pw2 = pps.tile([128, 128], F32, tag="pdg")
                        nc.tensor.matmul(pw2, Vprev, Wk[k], start=True, stop=True)
                        nc.vector.tensor_copy(Wk[k + 1], pw2)
                        if k < 5:
                            pv2 = pps.tile([128, 128], F32, tag="pdg")
                            nc.tensor.matmul(pv2, Wk[k], Vprev, start=True, stop=True)
                            Vnew = wvpool.tile([128, 128], BF16, tag="V")
                            nc.vector.tensor_copy(Vnew, pv2)
                            Vprev = Vnew

                    # wide kq row for blocks > j (raw + exp)
                    wide = []
                    t0 = (j + 1) * 128
                    while t0 < T:
                        tw = min(512, T - t0)
                        pkw = pp.tile([128, 512], F32, tag="psp")
                        nc.tensor.matmul(
                            pkw[:, :tw], ks, qTt[:, hb + t0: hb + t0 + tw],
                            start=True, stop=True)
                        raw = rawpool.tile([128, 512], BF16, tag="raw")
                        nc.vector.tensor_copy(raw[:, :tw], pkw[:, :tw])
                        Ew = epool.tile([128, 512], BF16, tag="ew")
                        nc.scalar.activation(Ew[:, :tw], pkw[:, :tw], EXP)
                        wide.append((t0, tw, raw, Ew))
                        t0 += tw

                    # c_j = v_j - beta * acc_j   (acc transposed back to rows)
                    pat = pps.tile([128, 128], F32, tag="pdg")
                    nc.tensor.transpose(pat, accT[:, jb:jb + 128], ident)
                    ctmp = tfpool.tile([128, 128], F32, tag="tmpf")
                    nc.scalar.activation(
                        ctmp, pat, CPY, scale=nbeta[:, h * NB + j: h * NB + j + 1])
                    cf = tfpool.tile([128, 128], F32, tag="tmpf")
                    nc.vector.tensor_tensor(cf, ctmp, vt[:, jb:jb + 128], op=ADD)
                    wprev = wapool.tile([128, 128], BF16, tag="wap")
                    nc.vector.tensor_copy(wprev, cf)
                    wprevf = cf

                    # apply (I+Y)(I+Y^2)...(I+Y^64) c
                    for k in range(6, -1, -1):
                        pw = ppw.tile([128, 128], F32, tag="pw")
                        nc.tensor.matmul(pw, Wk[k], wprev, start=True, stop=True)
                        if k > 0:
                            wnf = tfpool.tile([128, 128], F32, tag="tmpf")
                            nc.vector.tensor_tensor(wnf, pw, wprevf, op=ADD)
                            wnew = wapool.tile([128, 128], BF16, tag="wap")
                            nc.vector.tensor_copy(wnew, wnf)
                            wprev, wprevf = wnew, wnf
                        else:
                            nc.vector.tensor_tensor(
                                ut[:, jb:jb + 128], pw, wprevf, op=ADD)
                    us = ut[:, jb:jb + 128]

                    # diagonal attention contributions
                    po = pps.tile([128, 128], F32, tag="pdg")
                    nc.tensor.matmul(po, us, Ejj, start=True, stop=True)
                    nc.vector.tensor_tensor(
                        oT[:, jb:jb + 128], oT[:, jb:jb + 128], po, op=ADD)
                    pd = ppd.tile([1, 512], F32, tag="pd")
                    nc.tensor.matmul(pd[:, :128], ones1, Ejj, start=True, stop=True)
                    nc.vector.tensor_tensor(
                        den[0:1, jb:jb + 128], den[0:1, jb:jb + 128],
                        pd[:, :128], op=ADD)

                    # wide updates: corrections (raw) + attention (E)
                    for (t0, tw, raw, Ew) in wide:
                        sl = slice(hb + t0, hb + t0 + tw)
                        pb = pp.tile([128, 512], F32, tag="psp")
                        nc.tensor.matmul(pb[:, :tw], us, raw[:, :tw],
                                         start=True, stop=True)
                        nc.vector.tensor_tensor(
                            accT[:, sl], accT[:, sl], pb[:, :tw], op=ADD)
                        pc = pp.tile([128, 512], F32, tag="psp")
                        nc.tensor.matmul(pc[:, :tw], us, Ew[:, :tw],
                                         start=True, stop=True)
                        nc.vector.tensor_tensor(
                            oT[:, sl], oT[:, sl], pc[:, :tw], op=ADD)
                        pd2 = ppd.tile([1, 512], F32, tag="pd")
                        nc.tensor.matmul(pd2[:, :tw], ones1, Ew[:, :tw],
                                         start=True, stop=True)
                        nc.vector.tensor_tensor(
                            den[0:1, sl], den[0:1, sl], pd2[:, :tw], op=ADD)

            # dense-softmax pollution term: u_last^T @ 0 (nan iff u overflowed)
            for h in range(2):
                pp0 = pps.tile([128, 128], F32, tag="pdg")
                nc.tensor.matmul(
                    pp0, ut[:, h * T + T - 128: h * T + T],
                    zeroW[:, :128], start=True, stop=True)
                nc.vector.tensor_copy(polsb[0:1, h:h + 1], pp0[0:1, 0:1])
            nc.sync.dma_start(out=pol_d[:, :], in_=polsb)

            # ---------------- phase 3: normalize + output projection -------
            with tc.tile_pool(name="fin", bufs=2) as fin, \
                 tc.tile_pool(name="wop", bufs=1) as wop:
                for h in range(2):
                    hb = h * T
                    nc.vector.reciprocal(
                        den[0:1, hb:hb + T], den[0:1, hb:hb + T])
                    rb = fin.tile([128, T], F32, tag="rb")
                    nc.gpsimd.partition_broadcast(rb, den[0:1, hb:hb + T])
                    nc.vector.tensor_tensor(
                        oTb[:, hb:hb + T], oT[:, hb:hb + T], rb, op=MUL)

                wosb = wop.tile([128, 2 * T], BF16, tag="wob")
                for h in range(2):
                    nc.sync.dma_start(
                        out=wosb[:, h * T:(h + 1) * T],
                        in_=wo_d[h * 128:(h + 1) * 128, :])
                ypart = dp.tile([T, HID], BF16, tag="ypart")
                for tb in range(NB):
                    ys = fin.tile([128, HID], BF16, tag="ys")
                    for nch in range(4):
                        py = pp.tile([128, 512], F32, tag="psp")
                        for h in range(2):
                            nc.tensor.matmul(
                                py,
                                oTb[:, h * T + tb * 128: h * T + tb * 128 + 128],
                                wosb[:, h * T + nch * 512: h * T + (nch + 1) * 512],
                                start=(h == 0), stop=(h == 1))
                        nc.scalar.activation(
                            ys[:, nch * 512:(nch + 1) * 512], py, CPY)
                    nc.sync.dma_start(
                        out=ypart[tb * 128:(tb + 1) * 128, :], in_=ys)
                # sum the 8 partial products on-device; each core keeps 1/8
                yrs = dp.tile([T // 8, HID], BF16, tag="yrs")
                nc.gpsimd.collective_compute(
                    "ReduceScatter", ADD, replica_groups=[list(range(8))],
                    ins=[ypart[:, :]], outs=[yrs[:, :]])
                nc.sync.dma_start(out=y_d[:, :], in_=yrs)

    nc.finalize()
    return nc


def _get_nc():
    global _NC
    if _NC is None:
        _NC = _build()
    return _NC


def kernel(**inputs):
    x = np.asarray(inputs["hidden_states"], np.float32)[0]
    Wq = np.asarray(inputs["Wq"], np.float32)
    Wk = np.asarray(inputs["Wk"], np.float32)
    Wv = np.asarray(inputs["Wv"], np.float32)
    Wb = np.asarray(inputs["Wb"], np.float32)
    bb = np.asarray(inputs["bb"], np.float32)
    Wo = np.asarray(inputs["Wo"], np.float32)

    import ml_dtypes
    bf16 = ml_dtypes.bfloat16
    # one transpose+cast per matrix (astype emits C-order), then cheap slices
    xTn = x.T.astype(bf16)
    WqT = Wq.T.astype(bf16)
    WkT = Wk.T.astype(bf16)
    WvT = Wv.T.astype(bf16)
    WbT = Wb.T.astype(bf16)
    WoT = Wo.T.astype(bf16)
    in_maps = []
    for c in range(8):
        r0 = c * 256
        in_maps.append({
            "xs": xTn[r0:r0 + 256],
            "wq": np.ascontiguousarray(WqT[:, r0:r0 + 256]),
            "wk": np.ascontiguousarray(WkT[:, r0:r0 + 256]),
            "wvb": np.concatenate([WvT[:, r0:r0 + 256],
                                   WbT[:, 2 * c:2 * c + 2]], axis=1),
            "wo": WoT[r0:r0 + 256],
            "bb2": np.ascontiguousarray(bb[2 * c:2 * c + 2].reshape(1, 2)),
        })

    from concourse.bass_utils import run_bass_kernel_spmd
    res = run_bass_kernel_spmd(_get_nc(), in_maps, core_ids=list(range(8)))
    globals()["_LAST_RES"] = res

    Y = np.concatenate([r["y"] for r in res.results], axis=0).astype(np.float32)
    polsum = np.float32(0.0)
    for r in res.results:
        polsum = polsum + r["pol"][0, 0] + r["pol"][0, 1]
    Y = Y + polsum
    return Y.reshape(1, T, HID)
